# revision 48
# baseline (speedup 1.0000x reference)
"""DeformableConv1d TRN2 Bass kernel (v4).

Per batch sample (one NeuronCore each, 8 cores):
  offset/mask = conv1d over x.T; pos = clip(l+off); fl/alpha; out[c,l] =
  sum_k mask*((1-a)*x[fl,c] + a*x[fl+1,c]) -- collapses to a 7-diagonal
  band: out[c,l] = sum_{s=-3..3} vv_s[l] * x[l+s, c].

v4 (vs v2 baseline):
 - PE-heavy path in bf16 (fp32r matmuls run 4 cycles/row on HW; bf16 runs
   1): x tiles are cast to bf16 as they arrive, transposes/conv/band all
   stream bf16. The f32 floor/sigmoid elementwise math is unchanged.
 - The masked M_ALL build (~100us of DVE+GpSimd in v2) is replaced by PE
   shift-matmuls into w2m[p, m*8+u] plus a DRAM bounce: w2m is scattered
   into a skewed DRAM image at flat p*(DSK_W+1) + m*134 + u, and read
   back with row stride DSK_W+1 into a NORMAL m_all AP, landing row p
   shifted by +p columns (SBUF APs cannot express diagonals; DRAM APs
   are plain linear). Non-diagonal entries stay zero via an early
   zero-image DMA sourced from the memset m_all region.
 - band: per (m, g): psum[c, 134] = x16_m[:,g].T @ m_all[:, m*134:+134];
   drains rotate vector/scalar/gpsimd, seams on vector; 6 PSUM bufs.
 - stores are quartered and issued as out_cl columns finalize.
"""
import numpy as np
from contextlib import ExitStack

import bass_rust
import ml_dtypes
import concourse.bacc as bacc
import concourse.bass as bass
import concourse.tile as tile
from concourse import mybir
from concourse.bass_utils import run_bass_kernel_spmd

AP = bass_rust.AP
dt = mybir.dt
F32 = dt.float32
F32R = dt.float32r
BF16 = dt.bfloat16
BFNP = ml_dtypes.bfloat16

B, L, C, K = 8, 4096, 256, 3
P = 128
NT = L // P            # 32 aligned l-tiles
ND = 7                 # diagonals s in [-3, 3]
F = 134                # band free width per tile: f in [0,134), l = 128m-3+f
XT_W = L + 2           # xT padded with a zero col at l=-1 and l=L
MW = NT * F + F        # m_all width + slack (nothing reads the slack now)
W2W = NT * 8           # w2m width: 8 slots per m (7 used)
DSK_W = NT * F + 1     # skewed DRAM image row pitch (+1 gives the shift)
_cache = {}


def _build(w_off, b_off, w_mask, b_mask):
    nc = bacc.Bacc("TRN2", target_bir_lowering=False, debug=False)

    x_in = nc.dram_tensor("x", [L, C], F32, kind="ExternalInput").ap()
    out_d = nc.dram_tensor("out", [C, L], F32, kind="ExternalOutput").ap()
    mskew_d = nc.dram_tensor("mskew", [P, DSK_W], BF16, kind="Internal").ap()

    # conv weights [c-in-group, (g, dk, j)]; j<3 offset o, j>=3 mask o
    wcat = np.zeros((P, 36), np.float32)
    for g in range(2):
        for dkk in range(3):
            for j in range(6):
                w = w_off if j < 3 else w_mask
                wcat[:, g * 18 + dkk * 6 + j] = w[j % 3, g * P:(g + 1) * P, dkk]
    wcat_h = nc.inline_tensor(np.ascontiguousarray(wcat.astype(BFNP)),
                              name="wcat")
    ident_h = nc.inline_tensor(np.eye(P, dtype=np.float32), name="ident")
    ident6_h = nc.inline_tensor(np.eye(6, dtype=np.float32), name="ident6")

    # shift matrices: main SH_u[k,p]=1[k=p+u-3]; carries for tile wrap
    shmats = {}
    for u in range(ND):
        sh = u - 3
        m_ = np.zeros((P, P), np.float32)
        for p in range(P):
            if 0 <= p + sh < P:
                m_[p + sh, p] = 1.0
        shmats[("m", u)] = m_
        if sh > 0:
            c_ = np.zeros((P, P), np.float32)
            for p in range(P - sh, P):
                c_[p + sh - P, p] = 1.0
            shmats[("c", u)] = c_
        elif sh < 0:
            c_ = np.zeros((P, P), np.float32)
            for p in range(0, -sh):
                c_[p + sh + P, p] = 1.0
            shmats[("c", u)] = c_
    sh_h = {k: nc.inline_tensor(np.ascontiguousarray(v.astype(BFNP)),
                                name=f"sh_{k[0]}{k[1]}")
            for k, v in shmats.items()}

    bo = [float(v) for v in np.asarray(b_off)]
    bm = [float(v) for v in np.asarray(b_mask)]
    A = mybir.AluOpType

    with tile.TileContext(nc) as tc, ExitStack() as ctx:
        pool = ctx.enter_context(tc.tile_pool(name="main", bufs=1))
        ctx2 = ctx.enter_context(ExitStack())
        ps_tr = ctx2.enter_context(tc.tile_pool(name="ps_tr", bufs=3, space="PSUM"))
        ps_cv = ctx2.enter_context(tc.tile_pool(name="ps_cv", bufs=2, space="PSUM"))
        ps_sh = ctx2.enter_context(tc.tile_pool(name="ps_sh", bufs=1, space="PSUM"))

        # ---- consts needed early go first on the gpsimd DMA queue ----
        ident_s = pool.tile([P, P], BF16, tag="ident")
        nc.gpsimd.dma_start(ident_s[:], ident_h.ap())
        wcat_s = pool.tile([P, 36], BF16, tag="wcat")
        nc.gpsimd.dma_start(wcat_s[:], wcat_h.ap())
        ident6_s = pool.tile([6, 6], F32, tag="ident6")
        nc.gpsimd.dma_start(ident6_s[:], ident6_h.ap())

        # ---- x tiles across all three DMA queues ----
        xal = [pool.tile([P, C], F32, tag=f"xal{m}", name=f"xal{m}")
               for m in range(NT)]
        dma_engs = [nc.sync, nc.gpsimd, nc.scalar]
        for m in range(NT):
            dma_engs[m % 3].dma_start(xal[m][:], x_in[m * P:(m + 1) * P, :])

        m_all = pool.tile([P, MW], BF16, tag="m_all")
        half = (MW // 2) & ~15
        nc.vector.memset(m_all[:, 0:half], 0.0)
        nc.gpsimd.memset(m_all[:, half:MW], 0.0)
        # zero the skewed DRAM image from the just-zeroed m_all region
        nc.gpsimd.dma_start(mskew_d, AP(m_all[:].tensor, 0, [[MW, P], [1, DSK_W]]))

        # remaining consts (not needed until the shift phase) on sync queue
        sh_s = {}
        for kk, h in sh_h.items():
            t_ = pool.tile([P, P], BF16, tag=f"sh_{kk[0]}{kk[1]}",
                           name=f"sh_{kk[0]}{kk[1]}")
            nc.sync.dma_start(t_[:], h.ap())
            sh_s[kk] = t_

        # ---- per tile: cast to bf16, transpose both c-groups ----
        xT = [pool.tile([P, XT_W], BF16, tag=f"xT{g}", name=f"xT{g}")
              for g in range(2)]
        for g in range(2):
            nc.vector.memset(xT[g][:, 0:1], 0.0)
            nc.vector.memset(xT[g][:, XT_W - 1:XT_W], 0.0)
        x16b = pool.tile([P, NT * C], BF16, tag="x16b")
        for m in range(NT):
            x16m = x16b[:, m * C:(m + 1) * C]
            if m % 2 == 0:
                nc.vector.tensor_copy(x16m, xal[m][:])
            else:
                nc.scalar.copy(x16m, xal[m][:])
            for g in range(2):
                pt = ps_tr.tile([P, P], BF16, tag="pt")
                nc.tensor.transpose(pt[:], x16b[:, m * C + g * P: m * C + (g + 1) * P],
                                    ident_s[:])
                dst = xT[g][:, 1 + m * P: 1 + (m + 1) * P]
                if (m + g) % 2 == 0:
                    nc.scalar.copy(dst, pt[:])
                else:
                    nc.vector.tensor_copy(dst, pt[:])

        # ---- conv -> z6 [6, L]; zT6 transposes interleave per chunk ----
        z6 = pool.tile([6, L], F32, tag="z6")
        zT6 = pool.tile([P, NT * 6], F32, tag="zT6")
        for chk in range(8):
            pz = ps_cv.tile([6, 512], F32, tag="pz")
            n = 0
            for g in range(2):
                for dkk in range(3):
                    lhsT = wcat_s[:, g * 18 + dkk * 6: g * 18 + dkk * 6 + 6]
                    rhs = xT[g][:, chk * 512 + dkk: chk * 512 + dkk + 512]
                    nc.tensor.matmul(pz[:], lhsT, rhs, start=(n == 0), stop=(n == 5))
                    n += 1
            nc.scalar.copy(z6[:, chk * 512:(chk + 1) * 512], pz[:])
            for m in (4 * chk, 4 * chk + 1, 4 * chk + 2, 4 * chk + 3):
                pzt = ps_tr.tile([P, 6], F32, tag="pt")
                nc.tensor.transpose(pzt[:], z6[:, m * P:(m + 1) * P], ident6_s[:])
                nc.vector.tensor_copy(zT6[:, m * 6:(m + 1) * 6], pzt[:])

        # ---- elementwise -> d/wf/wc per offset row o ----
        iota = pool.tile([P, NT], F32, tag="iota")
        nc.gpsimd.iota(iota[:], pattern=[[P, NT]], base=0, channel_multiplier=1,
                       allow_small_or_imprecise_dtypes=True)
        spat = pool.tile([P, 9], F32, tag="spat")
        nc.gpsimd.iota(spat[:], pattern=[[1, 9]], base=-4, channel_multiplier=0,
                       allow_small_or_imprecise_dtypes=True)

        zt_h = zT6[:].tensor
        dts, wfs, wcs = [], [], []
        for o in range(3):
            off_o = AP(zt_h, o, [[NT * 6, P], [6, NT]])
            mlg_o = AP(zt_h, 3 + o, [[NT * 6, P], [6, NT]])
            pos = pool.tile([P, NT], F32, tag=f"pos{o}")
            nc.vector.scalar_tensor_tensor(pos[:], off_o, bo[o], iota[:],
                                           A.add, A.add)
            nc.vector.tensor_scalar(pos[:], pos[:], 0.0, float(L - 1), A.max, A.min)
            # floor via RNE(+-2^23) then fix up: fl = rne - (rne > pos)
            fl = pool.tile([P, NT], F32, tag=f"fl{o}")
            nc.vector.tensor_scalar(fl[:], pos[:], 8388608.0, 8388608.0,
                                    A.add, A.subtract)
            gt = pool.tile([P, NT], F32, tag=f"gt{o}")
            nc.vector.tensor_tensor(gt[:], fl[:], pos[:], A.is_gt)
            nc.vector.tensor_tensor(fl[:], fl[:], gt[:], A.subtract)
            alp = pool.tile([P, NT], F32, tag=f"alp{o}")
            nc.vector.tensor_tensor(alp[:], pos[:], fl[:], A.subtract)
            dd = pool.tile([P, NT], F32, tag=f"dd{o}")
            nc.vector.tensor_tensor(dd[:], fl[:], iota[:], A.subtract)
            msk = pool.tile([P, NT], F32, tag=f"msk{o}")
            nc.vector.tensor_scalar(msk[:], mlg_o, bm[o], None, A.add)
            nc.scalar.activation(msk[:], msk[:],
                                 mybir.ActivationFunctionType.Sigmoid)
            wc = pool.tile([P, NT], F32, tag=f"wc{o}")
            nc.vector.tensor_tensor(wc[:], msk[:], alp[:], A.mult)
            wf = pool.tile([P, NT], F32, tag=f"wf{o}")
            nc.vector.tensor_tensor(wf[:], msk[:], wc[:], A.subtract)
            dts.append(dd); wfs.append(wf); wcs.append(wc)

        # ---- VV2 [p, si*NT + t]: vv_{si-3}[t*128+p] ----
        vv2 = pool.tile([P, ND * NT], F32, tag="vv2")
        vv2_3d = AP(vv2[:].tensor, 0, [[ND * NT, P], [NT, ND], [1, NT]])
        eq = pool.tile([P, ND * NT], F32, tag="eq")
        eq_3d = AP(eq[:].tensor, 0, [[ND * NT, P], [NT, ND], [1, NT]])
        spat_f = AP(spat[:].tensor, 1, [[9, P], [1, ND], [0, NT]])  # si-3
        spat_c = AP(spat[:].tensor, 0, [[9, P], [1, ND], [0, NT]])  # si-4
        first = True
        for o in range(3):
            d3 = AP(dts[o][:].tensor, 0, [[NT, P], [0, ND], [1, NT]])
            wf3 = AP(wfs[o][:].tensor, 0, [[NT, P], [0, ND], [1, NT]])
            wc3 = AP(wcs[o][:].tensor, 0, [[NT, P], [0, ND], [1, NT]])
            for sp, w3 in ((spat_f, wf3), (spat_c, wc3)):
                nc.vector.tensor_tensor(eq_3d, d3, sp, A.is_equal)
                if first:
                    nc.vector.tensor_tensor(vv2_3d, eq_3d, w3, A.mult)
                    first = False
                else:
                    nc.vector.tensor_tensor(eq_3d, eq_3d, w3, A.mult)
                    nc.vector.tensor_tensor(vv2_3d, vv2_3d, eq_3d, A.add)

        # ---- w2m [p, m*8 + u] = vv_{3-u}[128m + p + u - 3] (PE shifts) ----
        vv2b = pool.tile([P, ND * NT], BF16, tag="vv2b")
        nc.vector.tensor_copy(vv2b[:], vv2[:])
        w2m = pool.tile([P, W2W], BF16, tag="w2m")
        w2m_h = w2m[:].tensor
        for u in range(ND):
            si = 6 - u
            sh = u - 3
            pw = ps_sh.tile([P, NT], F32, tag="pw")
            main_rhs = vv2b[:, si * NT:(si + 1) * NT]
            if sh == 0:
                nc.tensor.matmul(pw[:], sh_s[("m", u)][:], main_rhs,
                                 start=True, stop=True)
            elif sh > 0:
                nc.tensor.matmul(pw[:], sh_s[("m", u)][:], main_rhs,
                                 start=True, stop=False)
                nc.tensor.matmul(pw[:, 0:NT - 1], sh_s[("c", u)][:],
                                 vv2b[:, si * NT + 1:(si + 1) * NT],
                                 start=False, stop=True)
            else:
                nc.tensor.matmul(pw[:], sh_s[("m", u)][:], main_rhs,
                                 start=True, stop=False)
                nc.tensor.matmul(pw[:, 1:NT], sh_s[("c", u)][:],
                                 vv2b[:, si * NT:(si + 1) * NT - 1],
                                 start=False, stop=True)
            # strided drain: u contiguous within each m block of 8
            dst_u = AP(w2m_h, u, [[W2W, P], [8, NT]])
            nc.vector.tensor_copy(dst_u, pw[:])

        # ---- M_ALL[p, m*134 + p + u] = w2m[p, m*8 + u] via DRAM bounce ----
        # Step 1 scatters w2m into the skewed DRAM image at flat
        # p*(DSK_W+1) + m*134 + u; step 2 reads rows back with row stride
        # DSK_W+1 into a NORMAL m_all AP, landing row p shifted +p cols.
        # Quartered so early band matmuls start before late quarters land.
        # xhL rebases the bottom-3 halo rows to partitions 0..2 (PE matmul
        # operands must start at partition 0/32/64)
        xhL = pool.tile([3, NT * C], BF16, tag="xhL")
        nc.gpsimd.dma_start(
            AP(xhL[:].tensor, 0, [[NT * C, 3], [1, (NT - 1) * C]]),
            AP(x16b[:].tensor, 125 * NT * C, [[NT * C, 3], [1, (NT - 1) * C]]))
        # MHL rebases the left-halo seam columns of m_all; per quarter so
        # early band matmuls are not gated on the last bounce quarter
        mhl = pool.tile([3, NT * 3], BF16, tag="mhl")
        mskew_h = mskew_d.tensor
        m_h = m_all[:].tensor
        for q in range(4):
            dst1 = AP(mskew_h, 8 * q * F, [[DSK_W + 1, P], [F, 8], [1, ND]])
            src1 = AP(w2m_h, 8 * q * 8, [[W2W, P], [8, 8], [1, ND]])
            eng = nc.sync if q % 2 == 0 else nc.scalar
            eng.dma_start(dst1, src1)
            src2 = AP(mskew_h, 8 * q * F, [[DSK_W, P], [1, 8 * F]])
            dst2 = AP(m_all[:].tensor, 8 * q * F, [[MW, P], [1, 8 * F]])
            eng2 = nc.scalar if q % 2 == 0 else nc.sync
            eng2.dma_start(dst2, src2)
            m0 = max(1, 8 * q)
            cnt = 8 * (q + 1) - m0
            eng.dma_start(
                AP(mhl[:].tensor, m0 * 3, [[NT * 3, 3], [3, cnt], [1, 3]]),
                AP(m_h, 125 * MW + (m0 - 1) * F + 131, [[MW, 3], [F, cnt], [1, 3]]))

        # ---- band matmuls (halo scheme, no seam adds) into out_CL ----
        # psum[c, fo] covers out l = 128m + fo exactly. Main MM contracts
        # x16[m]; two 3-row halo MMs pull the cross-tile taps from the
        # neighboring M_ALL blocks' seam columns, accumulating in PSUM.
        ctx2.close()  # release ps_tr/ps_cv/ps_sh banks for the band pool
        ps_bd = ctx.enter_context(tc.tile_pool(name="ps_bd", bufs=8, space="PSUM"))
        out_cl = [pool.tile([P, L], F32, tag=f"ocl{g}", name=f"ocl{g}")
                  for g in range(2)]
        for m in range(NT):
            for g in range(2):
                pb = ps_bd.tile([P, P], F32, tag="pb")
                rhs = AP(m_h, m * F + 3, [[MW, P], [1, P]])
                nc.tensor.matmul(pb[:], x16b[:, m * C + g * P: m * C + (g + 1) * P],
                                 rhs, start=True, stop=False)
                if m > 0:
                    # left halo: x rows 128m-3..128m-1 -> out fo in [0, 3)
                    nc.tensor.matmul(
                        pb[:, 0:3],
                        xhL[0:3, (m - 1) * C + g * P:(m - 1) * C + (g + 1) * P],
                        mhl[0:3, m * 3:(m + 1) * 3],
                        start=False, stop=(m == NT - 1))
                if m < NT - 1:
                    # right halo: x rows 128(m+1)..+2 -> out fo in [125, 128)
                    nc.tensor.matmul(
                        pb[:, 125:128],
                        AP(x16b[:].tensor, (m + 1) * C + g * P,
                           [[NT * C, 3], [1, P]]),
                        AP(m_h, (m + 1) * F, [[MW, 3], [1, 3]]),
                        start=False, stop=True)
                dst = out_cl[g][:, m * P:(m + 1) * P]
                if (m + g) % 2 == 0:
                    nc.scalar.copy(dst, pb[:])
                else:
                    nc.vector.tensor_copy(dst, pb[:])
            # quartered stores: cols [0, 1024(k+1)) final once iter 8k+7 done
            if m in (7, 15, 23):
                h = (m + 1) // 8 - 1
                for g in range(2):
                    eng = nc.sync if (h + g) % 2 == 0 else nc.scalar
                    eng.dma_start(
                        out_d[g * P:(g + 1) * P, h * 1024:(h + 1) * 1024],
                        out_cl[g][:, h * 1024:(h + 1) * 1024])
        for g in range(2):
            eng = nc.sync if g % 2 == 0 else nc.scalar
            eng.dma_start(out_d[g * P:(g + 1) * P, 3072:4096],
                          out_cl[g][:, 3072:4096])

    nc.compile()
    return nc


def _get_nc(w_off, b_off, w_mask, b_mask):
    key = (w_off.tobytes(), b_off.tobytes(), w_mask.tobytes(), b_mask.tobytes())
    if key not in _cache:
        _cache[key] = _build(w_off, b_off, w_mask, b_mask)
    return _cache[key]


def kernel(x, w_off, b_off, w_mask, b_mask):
    x = np.ascontiguousarray(np.asarray(x, dtype=np.float32))
    nc = _get_nc(np.asarray(w_off, np.float32), np.asarray(b_off, np.float32),
                 np.asarray(w_mask, np.float32), np.asarray(b_mask, np.float32))
    in_maps = [{"x": x[b]} for b in range(B)]
    res = run_bass_kernel_spmd(nc, in_maps, list(range(B)))
    # out_d is the (C, L) buffer; reference returns its raw (L, C) reshape
    return np.stack([res.results[b]["out"].reshape(L, C) for b in range(B)])


# revision 49
# speedup vs baseline: 1.0590x; 1.0590x over previous
"""DeformableConv1d TRN2 Bass kernel (v4).

Per batch sample (one NeuronCore each, 8 cores):
  offset/mask = conv1d over x.T; pos = clip(l+off); fl/alpha; out[c,l] =
  sum_k mask*((1-a)*x[fl,c] + a*x[fl+1,c]) -- collapses to a 7-diagonal
  band: out[c,l] = sum_{s=-3..3} vv_s[l] * x[l+s, c].

v4 (vs v2 baseline):
 - PE-heavy path in bf16 (fp32r matmuls run 4 cycles/row on HW; bf16 runs
   1): x tiles are cast to bf16 as they arrive, transposes/conv/band all
   stream bf16. The f32 floor/sigmoid elementwise math is unchanged.
 - The masked M_ALL build (~100us of DVE+GpSimd in v2) is replaced by PE
   shift-matmuls into w2m[p, m*8+u] plus a DRAM bounce: w2m is scattered
   into a skewed DRAM image at flat p*(DSK_W+1) + m*134 + u, and read
   back with row stride DSK_W+1 into a NORMAL m_all AP, landing row p
   shifted by +p columns (SBUF APs cannot express diagonals; DRAM APs
   are plain linear). Non-diagonal entries stay zero via an early
   zero-image DMA sourced from the memset m_all region.
 - band: per (m, g): psum[c, 134] = x16_m[:,g].T @ m_all[:, m*134:+134];
   drains rotate vector/scalar/gpsimd, seams on vector; 6 PSUM bufs.
 - stores are quartered and issued as out_cl columns finalize.
"""
import numpy as np
from contextlib import ExitStack

import bass_rust
import ml_dtypes
import concourse.bacc as bacc
import concourse.bass as bass
import concourse.tile as tile
from concourse import mybir
from concourse.bass_utils import run_bass_kernel_spmd

AP = bass_rust.AP
dt = mybir.dt
F32 = dt.float32
F32R = dt.float32r
BF16 = dt.bfloat16
BFNP = ml_dtypes.bfloat16

B, L, C, K = 8, 4096, 256, 3
P = 128
NT = L // P            # 32 aligned l-tiles
ND = 7                 # diagonals s in [-3, 3]
F = 134                # band free width per tile: f in [0,134), l = 128m-3+f
XT_W = L + 2           # xT padded with a zero col at l=-1 and l=L
MW = NT * F + F        # m_all width + slack (nothing reads the slack now)
W2W = NT * 8           # w2m width: 8 slots per m (7 used)
DSK_W = NT * F + 1     # skewed DRAM image row pitch (+1 gives the shift)
_cache = {}


def _build(w_off, b_off, w_mask, b_mask):
    nc = bacc.Bacc("TRN2", target_bir_lowering=False, debug=False)

    x_in = nc.dram_tensor("x", [L, C], F32, kind="ExternalInput").ap()
    out_d = nc.dram_tensor("out", [C, L], F32, kind="ExternalOutput").ap()
    mskew_d = nc.dram_tensor("mskew", [P, DSK_W], BF16, kind="Internal").ap()

    # conv weights [c-in-group, (g, dk, j)]; j<3 offset o, j>=3 mask o
    wcat = np.zeros((P, 36), np.float32)
    for g in range(2):
        for dkk in range(3):
            for j in range(6):
                w = w_off if j < 3 else w_mask
                wcat[:, g * 18 + dkk * 6 + j] = w[j % 3, g * P:(g + 1) * P, dkk]
    wcat_h = nc.inline_tensor(np.ascontiguousarray(wcat.astype(BFNP)),
                              name="wcat")
    ident_h = nc.inline_tensor(np.eye(P, dtype=np.float32), name="ident")
    ident6_h = nc.inline_tensor(np.eye(6, dtype=np.float32), name="ident6")

    # shift matrices: main SH_u[k,p]=1[k=p+u-3]; carries for tile wrap
    shmats = {}
    for u in range(ND):
        sh = u - 3
        m_ = np.zeros((P, P), np.float32)
        for p in range(P):
            if 0 <= p + sh < P:
                m_[p + sh, p] = 1.0
        shmats[("m", u)] = m_
        if sh > 0:
            c_ = np.zeros((P, P), np.float32)
            for p in range(P - sh, P):
                c_[p + sh - P, p] = 1.0
            shmats[("c", u)] = c_
        elif sh < 0:
            c_ = np.zeros((P, P), np.float32)
            for p in range(0, -sh):
                c_[p + sh + P, p] = 1.0
            shmats[("c", u)] = c_
    sh_h = {k: nc.inline_tensor(np.ascontiguousarray(v.astype(BFNP)),
                                name=f"sh_{k[0]}{k[1]}")
            for k, v in shmats.items()}

    bo = [float(v) for v in np.asarray(b_off)]
    bm = [float(v) for v in np.asarray(b_mask)]
    A = mybir.AluOpType

    with tile.TileContext(nc) as tc, ExitStack() as ctx:
        pool = ctx.enter_context(tc.tile_pool(name="main", bufs=1))
        ctx2 = ctx.enter_context(ExitStack())
        ps_tr = ctx2.enter_context(tc.tile_pool(name="ps_tr", bufs=2, space="PSUM"))
        ps_cv = ctx2.enter_context(tc.tile_pool(name="ps_cv", bufs=2, space="PSUM"))
        ps_sh = ctx2.enter_context(tc.tile_pool(name="ps_sh", bufs=1, space="PSUM"))

        # ---- consts needed early go first on the gpsimd DMA queue ----
        ident_s = pool.tile([P, P], BF16, tag="ident")
        nc.gpsimd.dma_start(ident_s[:], ident_h.ap())
        wcat_s = pool.tile([P, 36], BF16, tag="wcat")
        nc.gpsimd.dma_start(wcat_s[:], wcat_h.ap())
        ident6_s = pool.tile([6, 6], F32, tag="ident6")
        nc.gpsimd.dma_start(ident6_s[:], ident6_h.ap())

        # ---- x tiles on sync+gpsimd queues (scalar kept for casts/drains) ----
        xal = [pool.tile([P, C], F32, tag=f"xal{m}", name=f"xal{m}")
               for m in range(NT)]
        for m in range(NT):
            eng = nc.sync if m % 2 == 0 else nc.gpsimd
            eng.dma_start(xal[m][:], x_in[m * P:(m + 1) * P, :])

        m_all = pool.tile([P, MW], BF16, tag="m_all")
        half = (MW // 2) & ~15
        nc.vector.memset(m_all[:, 0:half], 0.0)
        nc.gpsimd.memset(m_all[:, half:MW], 0.0)
        # zero the skewed DRAM image from the just-zeroed m_all region
        nc.gpsimd.dma_start(mskew_d, AP(m_all[:].tensor, 0, [[MW, P], [1, DSK_W]]))

        # remaining consts (not needed until the shift phase) on sync queue
        sh_s = {}
        for kk, h in sh_h.items():
            t_ = pool.tile([P, P], BF16, tag=f"sh_{kk[0]}{kk[1]}",
                           name=f"sh_{kk[0]}{kk[1]}")
            nc.sync.dma_start(t_[:], h.ap())
            sh_s[kk] = t_

        # ---- per tile: cast to bf16, transpose both c-groups ----
        xT = [pool.tile([P, XT_W], BF16, tag=f"xT{g}", name=f"xT{g}")
              for g in range(2)]
        for g in range(2):
            nc.vector.memset(xT[g][:, 0:1], 0.0)
            nc.vector.memset(xT[g][:, XT_W - 1:XT_W], 0.0)
        x16b = pool.tile([P, NT * C], BF16, tag="x16b")
        for m in range(NT):
            x16m = x16b[:, m * C:(m + 1) * C]
            if m % 2 == 0:
                nc.vector.tensor_copy(x16m, xal[m][:])
            else:
                nc.scalar.copy(x16m, xal[m][:])
            for g in range(2):
                pt = ps_tr.tile([P, P], BF16, tag="pt")
                nc.tensor.transpose(pt[:], x16b[:, m * C + g * P: m * C + (g + 1) * P],
                                    ident_s[:])
                dst = xT[g][:, 1 + m * P: 1 + (m + 1) * P]
                if (m + g) % 2 == 0:
                    nc.scalar.copy(dst, pt[:])
                else:
                    nc.vector.tensor_copy(dst, pt[:])

        # ---- conv -> z6 [6, L]; zT6 transposes interleave per chunk ----
        z6 = pool.tile([6, L], F32, tag="z6")
        zT6 = pool.tile([P, NT * 6], F32, tag="zT6")
        for chk in range(8):
            pz = ps_cv.tile([6, 512], F32, tag="pz")
            n = 0
            for g in range(2):
                for dkk in range(3):
                    lhsT = wcat_s[:, g * 18 + dkk * 6: g * 18 + dkk * 6 + 6]
                    rhs = xT[g][:, chk * 512 + dkk: chk * 512 + dkk + 512]
                    nc.tensor.matmul(pz[:], lhsT, rhs, start=(n == 0), stop=(n == 5))
                    n += 1
            nc.scalar.copy(z6[:, chk * 512:(chk + 1) * 512], pz[:])
            for m in (4 * chk, 4 * chk + 1, 4 * chk + 2, 4 * chk + 3):
                pzt = ps_tr.tile([P, 6], F32, tag="pt")
                nc.tensor.transpose(pzt[:], z6[:, m * P:(m + 1) * P], ident6_s[:])
                nc.vector.tensor_copy(zT6[:, m * 6:(m + 1) * 6], pzt[:])

        # ---- elementwise -> d/wf/wc per offset row o ----
        iota = pool.tile([P, NT], F32, tag="iota")
        nc.gpsimd.iota(iota[:], pattern=[[P, NT]], base=0, channel_multiplier=1,
                       allow_small_or_imprecise_dtypes=True)
        spat = pool.tile([P, 9], F32, tag="spat")
        nc.gpsimd.iota(spat[:], pattern=[[1, 9]], base=-4, channel_multiplier=0,
                       allow_small_or_imprecise_dtypes=True)

        zt_h = zT6[:].tensor
        dts, wfs, wcs = [], [], []
        for o in range(3):
            off_o = AP(zt_h, o, [[NT * 6, P], [6, NT]])
            mlg_o = AP(zt_h, 3 + o, [[NT * 6, P], [6, NT]])
            pos = pool.tile([P, NT], F32, tag=f"pos{o}")
            nc.vector.scalar_tensor_tensor(pos[:], off_o, bo[o], iota[:],
                                           A.add, A.add)
            nc.vector.tensor_scalar(pos[:], pos[:], 0.0, float(L - 1), A.max, A.min)
            # floor via RNE(+-2^23) then fix up: fl = rne - (rne > pos)
            fl = pool.tile([P, NT], F32, tag=f"fl{o}")
            nc.vector.tensor_scalar(fl[:], pos[:], 8388608.0, 8388608.0,
                                    A.add, A.subtract)
            gt = pool.tile([P, NT], F32, tag=f"gt{o}")
            nc.vector.tensor_tensor(gt[:], fl[:], pos[:], A.is_gt)
            nc.vector.tensor_tensor(fl[:], fl[:], gt[:], A.subtract)
            alp = pool.tile([P, NT], F32, tag=f"alp{o}")
            nc.vector.tensor_tensor(alp[:], pos[:], fl[:], A.subtract)
            dd = pool.tile([P, NT], F32, tag=f"dd{o}")
            nc.vector.tensor_tensor(dd[:], fl[:], iota[:], A.subtract)
            msk = pool.tile([P, NT], F32, tag=f"msk{o}")
            nc.vector.tensor_scalar(msk[:], mlg_o, bm[o], None, A.add)
            nc.scalar.activation(msk[:], msk[:],
                                 mybir.ActivationFunctionType.Sigmoid)
            wc = pool.tile([P, NT], F32, tag=f"wc{o}")
            nc.vector.tensor_tensor(wc[:], msk[:], alp[:], A.mult)
            wf = pool.tile([P, NT], F32, tag=f"wf{o}")
            nc.vector.tensor_tensor(wf[:], msk[:], wc[:], A.subtract)
            dts.append(dd); wfs.append(wf); wcs.append(wc)

        # ---- VV2 [p, si*NT + t]: vv_{si-3}[t*128+p] ----
        vv2 = pool.tile([P, ND * NT], F32, tag="vv2")
        vv2_3d = AP(vv2[:].tensor, 0, [[ND * NT, P], [NT, ND], [1, NT]])
        eq = pool.tile([P, ND * NT], F32, tag="eq")
        eq_3d = AP(eq[:].tensor, 0, [[ND * NT, P], [NT, ND], [1, NT]])
        spat_f = AP(spat[:].tensor, 1, [[9, P], [1, ND], [0, NT]])  # si-3
        spat_c = AP(spat[:].tensor, 0, [[9, P], [1, ND], [0, NT]])  # si-4
        first = True
        for o in range(3):
            d3 = AP(dts[o][:].tensor, 0, [[NT, P], [0, ND], [1, NT]])
            wf3 = AP(wfs[o][:].tensor, 0, [[NT, P], [0, ND], [1, NT]])
            wc3 = AP(wcs[o][:].tensor, 0, [[NT, P], [0, ND], [1, NT]])
            for sp, w3 in ((spat_f, wf3), (spat_c, wc3)):
                nc.vector.tensor_tensor(eq_3d, d3, sp, A.is_equal)
                if first:
                    nc.vector.tensor_tensor(vv2_3d, eq_3d, w3, A.mult)
                    first = False
                else:
                    nc.vector.tensor_tensor(eq_3d, eq_3d, w3, A.mult)
                    nc.vector.tensor_tensor(vv2_3d, vv2_3d, eq_3d, A.add)

        # ---- w2m [p, m*8 + u] = vv_{3-u}[128m + p + u - 3] (PE shifts) ----
        vv2b = pool.tile([P, ND * NT], BF16, tag="vv2b")
        nc.vector.tensor_copy(vv2b[:], vv2[:])
        w2m = pool.tile([P, W2W], BF16, tag="w2m")
        w2m_h = w2m[:].tensor
        for u in range(ND):
            si = 6 - u
            sh = u - 3
            pw = ps_sh.tile([P, NT], F32, tag="pw")
            main_rhs = vv2b[:, si * NT:(si + 1) * NT]
            if sh == 0:
                nc.tensor.matmul(pw[:], sh_s[("m", u)][:], main_rhs,
                                 start=True, stop=True)
            elif sh > 0:
                nc.tensor.matmul(pw[:], sh_s[("m", u)][:], main_rhs,
                                 start=True, stop=False)
                nc.tensor.matmul(pw[:, 0:NT - 1], sh_s[("c", u)][:],
                                 vv2b[:, si * NT + 1:(si + 1) * NT],
                                 start=False, stop=True)
            else:
                nc.tensor.matmul(pw[:], sh_s[("m", u)][:], main_rhs,
                                 start=True, stop=False)
                nc.tensor.matmul(pw[:, 1:NT], sh_s[("c", u)][:],
                                 vv2b[:, si * NT:(si + 1) * NT - 1],
                                 start=False, stop=True)
            # strided drain: u contiguous within each m block of 8
            dst_u = AP(w2m_h, u, [[W2W, P], [8, NT]])
            nc.vector.tensor_copy(dst_u, pw[:])

        # ---- M_ALL[p, m*134 + p + u] = w2m[p, m*8 + u] via DRAM bounce ----
        # Step 1 scatters w2m into the skewed DRAM image at flat
        # p*(DSK_W+1) + m*134 + u; step 2 reads rows back with row stride
        # DSK_W+1 into a NORMAL m_all AP, landing row p shifted +p cols.
        # Quartered so early band matmuls start before late quarters land.
        # xhL rebases the bottom-3 halo rows to partitions 0..2 (PE matmul
        # operands must start at partition 0/32/64)
        xhL = pool.tile([3, NT * C], BF16, tag="xhL")
        nc.gpsimd.dma_start(
            AP(xhL[:].tensor, 0, [[NT * C, 3], [1, (NT - 1) * C]]),
            AP(x16b[:].tensor, 125 * NT * C, [[NT * C, 3], [1, (NT - 1) * C]]))
        # MHL rebases the left-halo seam columns of m_all; per quarter so
        # early band matmuls are not gated on the last bounce quarter
        mhl = pool.tile([3, NT * 3], BF16, tag="mhl")
        mskew_h = mskew_d.tensor
        m_h = m_all[:].tensor
        for q in range(4):
            dst1 = AP(mskew_h, 8 * q * F, [[DSK_W + 1, P], [F, 8], [1, ND]])
            src1 = AP(w2m_h, 8 * q * 8, [[W2W, P], [8, 8], [1, ND]])
            eng = nc.sync if q % 2 == 0 else nc.scalar
            eng.dma_start(dst1, src1)
            src2 = AP(mskew_h, 8 * q * F, [[DSK_W, P], [1, 8 * F]])
            dst2 = AP(m_all[:].tensor, 8 * q * F, [[MW, P], [1, 8 * F]])
            eng2 = nc.scalar if q % 2 == 0 else nc.sync
            eng2.dma_start(dst2, src2)
            m0 = max(1, 8 * q)
            cnt = 8 * (q + 1) - m0
            eng.dma_start(
                AP(mhl[:].tensor, m0 * 3, [[NT * 3, 3], [3, cnt], [1, 3]]),
                AP(m_h, 125 * MW + (m0 - 1) * F + 131, [[MW, 3], [F, cnt], [1, 3]]))

        # ---- band matmuls (halo scheme, no seam adds) into out_CL ----
        # psum[c, fo] covers out l = 128m + fo exactly. Main MM contracts
        # x16[m]; two 3-row halo MMs pull the cross-tile taps from the
        # neighboring M_ALL blocks' seam columns, accumulating in PSUM.
        ctx2.close()  # release ps_tr/ps_cv/ps_sh banks for the band pool
        ps_bd = ctx.enter_context(tc.tile_pool(name="ps_bd", bufs=6, space="PSUM"))
        out_cl = [pool.tile([P, L], F32, tag=f"ocl{g}", name=f"ocl{g}")
                  for g in range(2)]
        for m in range(NT):
            for g in range(2):
                pb = ps_bd.tile([P, P], F32, tag="pb")
                rhs = AP(m_h, m * F + 3, [[MW, P], [1, P]])
                nc.tensor.matmul(pb[:], x16b[:, m * C + g * P: m * C + (g + 1) * P],
                                 rhs, start=True, stop=False)
                if m > 0:
                    # left halo: x rows 128m-3..128m-1 -> out fo in [0, 3)
                    nc.tensor.matmul(
                        pb[:, 0:3],
                        xhL[0:3, (m - 1) * C + g * P:(m - 1) * C + (g + 1) * P],
                        mhl[0:3, m * 3:(m + 1) * 3],
                        start=False, stop=(m == NT - 1))
                if m < NT - 1:
                    # right halo: x rows 128(m+1)..+2 -> out fo in [125, 128)
                    nc.tensor.matmul(
                        pb[:, 125:128],
                        AP(x16b[:].tensor, (m + 1) * C + g * P,
                           [[NT * C, 3], [1, P]]),
                        AP(m_h, (m + 1) * F, [[MW, 3], [1, 3]]),
                        start=False, stop=True)
                dst = out_cl[g][:, m * P:(m + 1) * P]
                if (m + g) % 2 == 0:
                    nc.scalar.copy(dst, pb[:])
                else:
                    nc.vector.tensor_copy(dst, pb[:])
            # quartered stores: cols [0, 1024(k+1)) final once iter 8k+7 done
            if m in (7, 15, 23):
                h = (m + 1) // 8 - 1
                for g in range(2):
                    eng = nc.sync if (h + g) % 2 == 0 else nc.scalar
                    eng.dma_start(
                        out_d[g * P:(g + 1) * P, h * 1024:(h + 1) * 1024],
                        out_cl[g][:, h * 1024:(h + 1) * 1024])
        for g in range(2):
            eng = nc.sync if g % 2 == 0 else nc.scalar
            eng.dma_start(out_d[g * P:(g + 1) * P, 3072:4096],
                          out_cl[g][:, 3072:4096])

    nc.compile()
    return nc


def _get_nc(w_off, b_off, w_mask, b_mask):
    key = (w_off.tobytes(), b_off.tobytes(), w_mask.tobytes(), b_mask.tobytes())
    if key not in _cache:
        _cache[key] = _build(w_off, b_off, w_mask, b_mask)
    return _cache[key]


def kernel(x, w_off, b_off, w_mask, b_mask):
    x = np.ascontiguousarray(np.asarray(x, dtype=np.float32))
    nc = _get_nc(np.asarray(w_off, np.float32), np.asarray(b_off, np.float32),
                 np.asarray(w_mask, np.float32), np.asarray(b_mask, np.float32))
    in_maps = [{"x": x[b]} for b in range(B)]
    res = run_bass_kernel_spmd(nc, in_maps, list(range(B)))
    # out_d is the (C, L) buffer; reference returns its raw (L, C) reshape
    return np.stack([res.results[b]["out"].reshape(L, C) for b in range(B)])


# revision 50
# speedup vs baseline: 1.0641x; 1.0048x over previous
"""DeformableConv1d TRN2 Bass kernel (v7).

Per batch sample (one NeuronCore each, 8 cores):
  offset/mask = conv1d over x.T; pos = clip(l+off); fl/alpha; out[c,l] =
  sum_k mask*((1-a)*x[fl,c] + a*x[fl+1,c]) -- collapses to a 7-diagonal
  band: out[c,l] = sum_{s=-3..3} vv_s[l] * x[l+s, c].

Structure (vs the v2 baseline, 178us -> ~115us traced):
 - PE-heavy path in bf16 (fp32r matmuls run 4 cycles/row on this HW
   regardless of width; bf16 runs 1): x tiles are cast to bf16 as they
   arrive (vector/scalar), transposes/conv/shifts/band all stream bf16.
   The f32 floor/sigmoid elementwise math is unchanged from v2.
 - The masked M_ALL build (~100us of DVE+GpSimd in v2) is replaced by PE
   shift-matmuls into w2m[p, m*8+u] plus a DRAM bounce: w2m is scattered
   into a skewed DRAM image at flat p*(DSK_W+1) + m*134 + u, and read
   back with row stride DSK_W into a NORMAL m_all AP, landing row p
   shifted by +p columns (SBUF APs cannot express diagonals -- the DGE
   drops the fractional partition step beyond 32 partitions and the
   verifier rejects nonzero offsets; DRAM APs are plain linear).
   Non-diagonal entries stay zero via an early zero-image DMA sourced
   from the memset m_all region. Quartered so early band matmuls start
   before late quarters land.
 - band (halo scheme, no seam adds): per (m, g) a [128,128] PSUM tile
   covers out l = 128m+fo exactly; the main MM contracts x16 tile m
   against m_all[:, m*134+3 : +131]; two 3-row halo MMs accumulate the
   cross-tile taps (operands rebased to partition 0 via xhL/MHL staging
   DMAs -- PE operands must start at partition 0/32/64). Drains are pure
   [128,128] copies alternating vector/scalar; 6 PSUM bufs.
 - stores are quartered and issued as out_cl columns finalize.
"""
import numpy as np
from contextlib import ExitStack

import bass_rust
import ml_dtypes
import concourse.bacc as bacc
import concourse.bass as bass
import concourse.tile as tile
from concourse import mybir
from concourse.bass_utils import run_bass_kernel_spmd

AP = bass_rust.AP
dt = mybir.dt
F32 = dt.float32
F32R = dt.float32r
BF16 = dt.bfloat16
BFNP = ml_dtypes.bfloat16

B, L, C, K = 8, 4096, 256, 3
P = 128
NT = L // P            # 32 aligned l-tiles
ND = 7                 # diagonals s in [-3, 3]
F = 134                # band free width per tile: f in [0,134), l = 128m-3+f
XT_W = L + 2           # xT padded with a zero col at l=-1 and l=L
MW = NT * F + F        # m_all width + slack (nothing reads the slack now)
W2W = NT * 8           # w2m width: 8 slots per m (7 used)
DSK_W = NT * F + 1     # skewed DRAM image row pitch (+1 gives the shift)
_cache = {}


def _build(w_off, b_off, w_mask, b_mask):
    nc = bacc.Bacc("TRN2", target_bir_lowering=False, debug=False)

    x_in = nc.dram_tensor("x", [L, C], F32, kind="ExternalInput").ap()
    out_d = nc.dram_tensor("out", [C, L], F32, kind="ExternalOutput").ap()
    mskew_d = nc.dram_tensor("mskew", [P, DSK_W], BF16, kind="Internal").ap()

    # conv weights [c-in-group, (g, dk, j)]; j<3 offset o, j>=3 mask o
    wcat = np.zeros((P, 36), np.float32)
    for g in range(2):
        for dkk in range(3):
            for j in range(6):
                w = w_off if j < 3 else w_mask
                wcat[:, g * 18 + dkk * 6 + j] = w[j % 3, g * P:(g + 1) * P, dkk]
    wcat_h = nc.inline_tensor(np.ascontiguousarray(wcat.astype(BFNP)),
                              name="wcat")
    ident_h = nc.inline_tensor(np.eye(P, dtype=np.float32), name="ident")
    ident6_h = nc.inline_tensor(np.eye(6, dtype=np.float32), name="ident6")

    # shift matrices: main SH_u[k,p]=1[k=p+u-3]; carries for tile wrap
    shmats = {}
    for u in range(ND):
        sh = u - 3
        m_ = np.zeros((P, P), np.float32)
        for p in range(P):
            if 0 <= p + sh < P:
                m_[p + sh, p] = 1.0
        shmats[("m", u)] = m_
        if sh > 0:
            c_ = np.zeros((P, P), np.float32)
            for p in range(P - sh, P):
                c_[p + sh - P, p] = 1.0
            shmats[("c", u)] = c_
        elif sh < 0:
            c_ = np.zeros((P, P), np.float32)
            for p in range(0, -sh):
                c_[p + sh + P, p] = 1.0
            shmats[("c", u)] = c_
    sh_h = {k: nc.inline_tensor(np.ascontiguousarray(v.astype(BFNP)),
                                name=f"sh_{k[0]}{k[1]}")
            for k, v in shmats.items()}

    bo = [float(v) for v in np.asarray(b_off)]
    bm = [float(v) for v in np.asarray(b_mask)]
    A = mybir.AluOpType

    with tile.TileContext(nc) as tc, ExitStack() as ctx:
        pool = ctx.enter_context(tc.tile_pool(name="main", bufs=1))
        ctx2 = ctx.enter_context(ExitStack())
        ps_tr = ctx2.enter_context(tc.tile_pool(name="ps_tr", bufs=2, space="PSUM"))
        ps_cv = ctx2.enter_context(tc.tile_pool(name="ps_cv", bufs=2, space="PSUM"))
        ps_sh = ctx2.enter_context(tc.tile_pool(name="ps_sh", bufs=1, space="PSUM"))

        # ---- consts needed early go first on the gpsimd DMA queue ----
        ident_s = pool.tile([P, P], BF16, tag="ident")
        nc.gpsimd.dma_start(ident_s[:], ident_h.ap())
        wcat_s = pool.tile([P, 36], BF16, tag="wcat")
        nc.gpsimd.dma_start(wcat_s[:], wcat_h.ap())
        ident6_s = pool.tile([6, 6], F32, tag="ident6")
        nc.gpsimd.dma_start(ident6_s[:], ident6_h.ap())

        # ---- x tiles on sync+gpsimd queues (scalar kept for casts/drains) ----
        xal = [pool.tile([P, C], F32, tag=f"xal{m}", name=f"xal{m}")
               for m in range(NT)]
        for m in range(NT):
            eng = nc.sync if m % 2 == 0 else nc.gpsimd
            eng.dma_start(xal[m][:], x_in[m * P:(m + 1) * P, :])

        m_all = pool.tile([P, MW], BF16, tag="m_all")
        half = (MW // 2) & ~15
        nc.vector.memset(m_all[:, 0:half], 0.0)
        nc.gpsimd.memset(m_all[:, half:MW], 0.0)
        # zero the skewed DRAM image from the just-zeroed m_all region
        nc.gpsimd.dma_start(mskew_d, AP(m_all[:].tensor, 0, [[MW, P], [1, DSK_W]]))

        # remaining consts (not needed until the shift phase) on sync queue
        sh_s = {}
        for kk, h in sh_h.items():
            t_ = pool.tile([P, P], BF16, tag=f"sh_{kk[0]}{kk[1]}",
                           name=f"sh_{kk[0]}{kk[1]}")
            nc.sync.dma_start(t_[:], h.ap())
            sh_s[kk] = t_

        # ---- per tile: cast to bf16, transpose both c-groups ----
        xT = [pool.tile([P, XT_W], BF16, tag=f"xT{g}", name=f"xT{g}")
              for g in range(2)]
        for g in range(2):
            nc.vector.memset(xT[g][:, 0:1], 0.0)
            nc.vector.memset(xT[g][:, XT_W - 1:XT_W], 0.0)
        x16b = pool.tile([P, NT * C], BF16, tag="x16b")
        for m in range(NT):
            x16m = x16b[:, m * C:(m + 1) * C]
            if m % 2 == 0:
                nc.vector.tensor_copy(x16m, xal[m][:])
            else:
                nc.scalar.copy(x16m, xal[m][:])
            for g in range(2):
                pt = ps_tr.tile([P, P], BF16, tag="pt")
                nc.tensor.transpose(pt[:], x16b[:, m * C + g * P: m * C + (g + 1) * P],
                                    ident_s[:])
                dst = xT[g][:, 1 + m * P: 1 + (m + 1) * P]
                if (m + g) % 2 == 0:
                    nc.scalar.copy(dst, pt[:])
                else:
                    nc.vector.tensor_copy(dst, pt[:])

        # ---- conv -> z6 [6, L]; zT6 transposes interleave per chunk ----
        z6 = pool.tile([6, L], F32, tag="z6")
        zT6 = pool.tile([P, NT * 6], F32, tag="zT6")
        for chk in range(8):
            pz = ps_cv.tile([6, 512], F32, tag="pz")
            n = 0
            for g in range(2):
                for dkk in range(3):
                    lhsT = wcat_s[:, g * 18 + dkk * 6: g * 18 + dkk * 6 + 6]
                    rhs = xT[g][:, chk * 512 + dkk: chk * 512 + dkk + 512]
                    nc.tensor.matmul(pz[:], lhsT, rhs, start=(n == 0), stop=(n == 5))
                    n += 1
            nc.scalar.copy(z6[:, chk * 512:(chk + 1) * 512], pz[:])
            for m in (4 * chk, 4 * chk + 1, 4 * chk + 2, 4 * chk + 3):
                pzt = ps_tr.tile([P, 6], F32, tag="pt")
                nc.tensor.transpose(pzt[:], z6[:, m * P:(m + 1) * P], ident6_s[:])
                nc.vector.tensor_copy(zT6[:, m * 6:(m + 1) * 6], pzt[:])

        # ---- elementwise -> d/wf/wc per offset row o ----
        iota = pool.tile([P, NT], F32, tag="iota")
        nc.gpsimd.iota(iota[:], pattern=[[P, NT]], base=0, channel_multiplier=1,
                       allow_small_or_imprecise_dtypes=True)
        spat = pool.tile([P, 9], F32, tag="spat")
        nc.gpsimd.iota(spat[:], pattern=[[1, 9]], base=-4, channel_multiplier=0,
                       allow_small_or_imprecise_dtypes=True)

        zt_h = zT6[:].tensor
        dts, wfs, wcs = [], [], []
        for o in range(3):
            off_o = AP(zt_h, o, [[NT * 6, P], [6, NT]])
            mlg_o = AP(zt_h, 3 + o, [[NT * 6, P], [6, NT]])
            pos = pool.tile([P, NT], F32, tag=f"pos{o}")
            nc.vector.scalar_tensor_tensor(pos[:], off_o, bo[o], iota[:],
                                           A.add, A.add)
            nc.vector.tensor_scalar(pos[:], pos[:], 0.0, float(L - 1), A.max, A.min)
            # floor via RNE(+-2^23) then fix up: fl = rne - (rne > pos)
            fl = pool.tile([P, NT], F32, tag=f"fl{o}")
            nc.vector.tensor_scalar(fl[:], pos[:], 8388608.0, 8388608.0,
                                    A.add, A.subtract)
            gt = pool.tile([P, NT], F32, tag=f"gt{o}")
            nc.vector.tensor_tensor(gt[:], fl[:], pos[:], A.is_gt)
            nc.vector.tensor_tensor(fl[:], fl[:], gt[:], A.subtract)
            alp = pool.tile([P, NT], F32, tag=f"alp{o}")
            nc.vector.tensor_tensor(alp[:], pos[:], fl[:], A.subtract)
            dd = pool.tile([P, NT], F32, tag=f"dd{o}")
            nc.vector.tensor_tensor(dd[:], fl[:], iota[:], A.subtract)
            msk = pool.tile([P, NT], F32, tag=f"msk{o}")
            nc.vector.tensor_scalar(msk[:], mlg_o, bm[o], None, A.add)
            nc.scalar.activation(msk[:], msk[:],
                                 mybir.ActivationFunctionType.Sigmoid)
            wc = pool.tile([P, NT], F32, tag=f"wc{o}")
            nc.vector.tensor_tensor(wc[:], msk[:], alp[:], A.mult)
            wf = pool.tile([P, NT], F32, tag=f"wf{o}")
            nc.vector.tensor_tensor(wf[:], msk[:], wc[:], A.subtract)
            dts.append(dd); wfs.append(wf); wcs.append(wc)

        # ---- VV2 [p, si*NT + t]: vv_{si-3}[t*128+p] ----
        vv2 = pool.tile([P, ND * NT], F32, tag="vv2")
        vv2_3d = AP(vv2[:].tensor, 0, [[ND * NT, P], [NT, ND], [1, NT]])
        eq = pool.tile([P, ND * NT], F32, tag="eq")
        eq_3d = AP(eq[:].tensor, 0, [[ND * NT, P], [NT, ND], [1, NT]])
        spat_f = AP(spat[:].tensor, 1, [[9, P], [1, ND], [0, NT]])  # si-3
        spat_c = AP(spat[:].tensor, 0, [[9, P], [1, ND], [0, NT]])  # si-4
        first = True
        for o in range(3):
            d3 = AP(dts[o][:].tensor, 0, [[NT, P], [0, ND], [1, NT]])
            wf3 = AP(wfs[o][:].tensor, 0, [[NT, P], [0, ND], [1, NT]])
            wc3 = AP(wcs[o][:].tensor, 0, [[NT, P], [0, ND], [1, NT]])
            for sp, w3 in ((spat_f, wf3), (spat_c, wc3)):
                nc.vector.tensor_tensor(eq_3d, d3, sp, A.is_equal)
                if first:
                    nc.vector.tensor_tensor(vv2_3d, eq_3d, w3, A.mult)
                    first = False
                else:
                    nc.vector.tensor_tensor(eq_3d, eq_3d, w3, A.mult)
                    nc.vector.tensor_tensor(vv2_3d, vv2_3d, eq_3d, A.add)

        # ---- w2m [p, m*8 + u] = vv_{3-u}[128m + p + u - 3] (PE shifts) ----
        vv2b = pool.tile([P, ND * NT], BF16, tag="vv2b")
        nc.vector.tensor_copy(vv2b[:], vv2[:])
        w2m = pool.tile([P, W2W], BF16, tag="w2m")
        w2m_h = w2m[:].tensor
        for u in range(ND):
            si = 6 - u
            sh = u - 3
            pw = ps_sh.tile([P, NT], F32, tag="pw")
            main_rhs = vv2b[:, si * NT:(si + 1) * NT]
            if sh == 0:
                nc.tensor.matmul(pw[:], sh_s[("m", u)][:], main_rhs,
                                 start=True, stop=True)
            elif sh > 0:
                nc.tensor.matmul(pw[:], sh_s[("m", u)][:], main_rhs,
                                 start=True, stop=False)
                nc.tensor.matmul(pw[:, 0:NT - 1], sh_s[("c", u)][:],
                                 vv2b[:, si * NT + 1:(si + 1) * NT],
                                 start=False, stop=True)
            else:
                nc.tensor.matmul(pw[:], sh_s[("m", u)][:], main_rhs,
                                 start=True, stop=False)
                nc.tensor.matmul(pw[:, 1:NT], sh_s[("c", u)][:],
                                 vv2b[:, si * NT:(si + 1) * NT - 1],
                                 start=False, stop=True)
            # strided drain: u contiguous within each m block of 8
            dst_u = AP(w2m_h, u, [[W2W, P], [8, NT]])
            nc.vector.tensor_copy(dst_u, pw[:])

        # ---- M_ALL[p, m*134 + p + u] = w2m[p, m*8 + u] via DRAM bounce ----
        # Step 1 scatters w2m into the skewed DRAM image at flat
        # p*(DSK_W+1) + m*134 + u; step 2 reads rows back with row stride
        # DSK_W+1 into a NORMAL m_all AP, landing row p shifted +p cols.
        # Quartered so early band matmuls start before late quarters land.
        # xhL rebases the bottom-3 halo rows to partitions 0..2 (PE matmul
        # operands must start at partition 0/32/64)
        xhL = pool.tile([3, NT * C], BF16, tag="xhL")
        nc.gpsimd.dma_start(
            AP(xhL[:].tensor, 0, [[NT * C, 3], [1, (NT - 1) * C]]),
            AP(x16b[:].tensor, 125 * NT * C, [[NT * C, 3], [1, (NT - 1) * C]]))
        # MHL rebases the left-halo seam columns of m_all; per quarter so
        # early band matmuls are not gated on the last bounce quarter
        mhl = pool.tile([3, NT * 3], BF16, tag="mhl")
        mskew_h = mskew_d.tensor
        m_h = m_all[:].tensor
        for q in range(4):
            dst1 = AP(mskew_h, 8 * q * F, [[DSK_W + 1, P], [F, 8], [1, ND]])
            src1 = AP(w2m_h, 8 * q * 8, [[W2W, P], [8, 8], [1, ND]])
            eng = nc.sync if q % 2 == 0 else nc.scalar
            eng.dma_start(dst1, src1)
            src2 = AP(mskew_h, 8 * q * F, [[DSK_W, P], [1, 8 * F]])
            dst2 = AP(m_all[:].tensor, 8 * q * F, [[MW, P], [1, 8 * F]])
            eng2 = nc.scalar if q % 2 == 0 else nc.sync
            eng2.dma_start(dst2, src2)
            m0 = max(1, 8 * q)
            cnt = 8 * (q + 1) - m0
            eng.dma_start(
                AP(mhl[:].tensor, m0 * 3, [[NT * 3, 3], [3, cnt], [1, 3]]),
                AP(m_h, 125 * MW + (m0 - 1) * F + 131, [[MW, 3], [F, cnt], [1, 3]]))

        # ---- band matmuls (halo scheme, no seam adds) into out_CL ----
        # psum[c, fo] covers out l = 128m + fo exactly. Main MM contracts
        # x16[m]; two 3-row halo MMs pull the cross-tile taps from the
        # neighboring M_ALL blocks' seam columns, accumulating in PSUM.
        ctx2.close()  # release ps_tr/ps_cv/ps_sh banks for the band pool
        ps_bd = ctx.enter_context(tc.tile_pool(name="ps_bd", bufs=6, space="PSUM"))
        out_cl = [pool.tile([P, L], F32, tag=f"ocl{g}", name=f"ocl{g}")
                  for g in range(2)]
        for m in range(NT):
            for g in range(2):
                pb = ps_bd.tile([P, P], F32, tag="pb")
                rhs = AP(m_h, m * F + 3, [[MW, P], [1, P]])
                nc.tensor.matmul(pb[:], x16b[:, m * C + g * P: m * C + (g + 1) * P],
                                 rhs, start=True, stop=False)
                if m > 0:
                    # left halo: x rows 128m-3..128m-1 -> out fo in [0, 3)
                    nc.tensor.matmul(
                        pb[:, 0:3],
                        xhL[0:3, (m - 1) * C + g * P:(m - 1) * C + (g + 1) * P],
                        mhl[0:3, m * 3:(m + 1) * 3],
                        start=False, stop=(m == NT - 1))
                if m < NT - 1:
                    # right halo: x rows 128(m+1)..+2 -> out fo in [125, 128)
                    nc.tensor.matmul(
                        pb[:, 125:128],
                        AP(x16b[:].tensor, (m + 1) * C + g * P,
                           [[NT * C, 3], [1, P]]),
                        AP(m_h, (m + 1) * F, [[MW, 3], [1, 3]]),
                        start=False, stop=True)
                dst = out_cl[g][:, m * P:(m + 1) * P]
                if (m + g) % 2 == 0:
                    nc.scalar.copy(dst, pb[:])
                else:
                    nc.vector.tensor_copy(dst, pb[:])
            # quartered stores: cols [0, 1024(k+1)) final once iter 8k+7 done
            if m in (7, 15, 23):
                h = (m + 1) // 8 - 1
                for g in range(2):
                    eng = nc.sync if (h + g) % 2 == 0 else nc.scalar
                    eng.dma_start(
                        out_d[g * P:(g + 1) * P, h * 1024:(h + 1) * 1024],
                        out_cl[g][:, h * 1024:(h + 1) * 1024])
        for g in range(2):
            eng = nc.sync if g % 2 == 0 else nc.scalar
            eng.dma_start(out_d[g * P:(g + 1) * P, 3072:4096],
                          out_cl[g][:, 3072:4096])

    nc.compile()
    return nc


def _get_nc(w_off, b_off, w_mask, b_mask):
    key = (w_off.tobytes(), b_off.tobytes(), w_mask.tobytes(), b_mask.tobytes())
    if key not in _cache:
        _cache[key] = _build(w_off, b_off, w_mask, b_mask)
    return _cache[key]


def kernel(x, w_off, b_off, w_mask, b_mask):
    x = np.ascontiguousarray(np.asarray(x, dtype=np.float32))
    nc = _get_nc(np.asarray(w_off, np.float32), np.asarray(b_off, np.float32),
                 np.asarray(w_mask, np.float32), np.asarray(b_mask, np.float32))
    in_maps = [{"x": x[b]} for b in range(B)]
    res = run_bass_kernel_spmd(nc, in_maps, list(range(B)))
    # out_d is the (C, L) buffer; reference returns its raw (L, C) reshape
    return np.stack([res.results[b]["out"].reshape(L, C) for b in range(B)])
